# revision 12
# baseline (speedup 1.0000x reference)
"""Trainium2 Bass kernel for nn_CascadeGradNetOURS (dense_mlp).

Math (reference):
    h = x @ W.T                       # (B, E), shared by all layers
    z = beta[0] * (h + b[0])
    for i in 0..6:
        z = beta[i+1]*(h + b[i+1]) + alpha[i]*relu(z)
    z = alpha[7] * relu(z)
    out = z @ W + bias_last           # (B, IN)

Device formulation (per core, batch-sharded 1024 rows, transposed layout
hT[e, b] so per-layer consts become per-PARTITION scalars).

Cascade state algebra (v2): track X_k = rho_k*z_k + phi_k in fp16 with
per-e (rho, phi) chosen so every op is expressible with 2-slot DVE
tensor_scalar, plain TT adds against a single evicted tensor T, or an
ACT relu — per-(layer, ec-pair) engine assignment is a free knob:

    T   = nu*h + mu            (ACT Identity eviction, scale/bias APs;
                                nu = rho1*beta0, fuses layer-1's affine)
    L1:   Ut_1 = ts(T, 0, t1; max, mult)        [DVE 4x]
          X_2  = T (+) Ut_1                     [TT, DVE 2x or GPSIMD]
    k=2..8 DVE-impl:
          P  = ts(X_k, A, B; mult, add)         [4x]
          Ut = ts(P, 0, t_k; max, mult)         [4x; signed post-scale!]
          X_{k+1} = T (+) Ut                    [TT]
    k=2..8 ACT-impl:
          U  = ACT relu(a_k*X_k + b_k)  ( = |alpha_{k-1}|*relu(z_k) )
          hb = ts(T, A_hb, B_hb; mult, add)     [4x]
          X_{k+1} = hb (+) U                    [TT]
    z_sb = |alpha_7|*relu(z_8)  (k=8 output; sign(alpha_7) folded in W2)

rho1 normalization bounds |X| (|beta0/betak| can reach 4e4 -> fp16 inf
otherwise).  Validated vs the fp32 oracle in numpy: rel ~ 5.5e-4.
"""

import os

os.environ.setdefault("MYCRO_LOCAL_CACHE", "1")

import numpy as np

import concourse.bacc as bacc
import concourse.bass as bass
import concourse.mybir as mybir
from concourse.tile import TileContext

N_CORES = 8
B, IN, E, L = 8192, 1024, 4096, 8
BC = B // N_CORES          # 1024 batch rows per core
NI = IN // 128             # 8 i-chunks
NE = E // 128              # 32 e-chunks
NPAIR = NE // 2            # 16 ec-pairs
F16 = mybir.dt.float16
F32 = mybir.dt.float32
NCONST = 32

GROUP = 4                  # e-chunks interleaved in the cascade pipeline

# ---- engine assignment knobs (tuned on measurements) ----
# ACT_PAIRS[k] = set of pair indices (0..15) using ACT-impl for layer k
# (k = 2..8); everything else DVE-impl. Layer 1 is always DVE.
ACT_PAIRS = {k: set(range(NPAIR)) for k in (3, 4, 5, 6, 7)}
ACT_PAIRS[2] = set(range(12))
# TT_GP[k] = set of pair indices whose layer-k TT add (k=1..7) runs on
# GPSIMD instead of DVE.
TT_GP = {}
N_WARMUP_MM = 36           # junk matmuls to ramp the PE p-state during DMA

W1ECS = 20                 # mm2 window-1 depth (overlapped under cascade)

_SEQ_ONLY = {
    "InstUnconditionalBranch",
    "InstCall",
    "InstISA",
}


def _act_pairs(k):
    return ACT_PAIRS.get(k, set())


def _legalize_waits(nc):
    """Datapath instructions carry exactly ONE semaphore wait slot in the
    64-byte ISA encoding (walrus errors on more). Engine sequencers execute
    their stream in order, so any extra waits can be hoisted onto single-wait
    NoOps inserted immediately before the capped instruction — semantically
    identical (all waits still complete before the instruction executes).
    For HWDGE DMAs prefer keeping a DMA-queue wait in-descriptor and hoist
    engine-sem waits to the sequencer."""
    import bass_rust

    uid = 0
    for bb in nc.m.functions[0].blocks:
        insts = bb.instructions  # live list
        newlist = []
        for i in insts:
            cls = i.__class__.__name__
            si = i.sync_info
            if cls in _SEQ_ONLY or si is None or len(si.on_wait) <= 1:
                newlist.append(i)
                continue
            waits = list(si.on_wait)
            if cls == "InstDMACopy":
                dmaw = [w for w in waits if w.ant_name.startswith("DMA")]
                keep = dmaw[-1] if dmaw else waits[-1]
            else:
                keep = waits[-1]
            rest = [w for w in waits if w is not keep]
            for w in rest:
                uid += 1
                nop = mybir.InstNoOp(
                    name=f"waitnop-{uid}-{i.name}",
                    engine=i.engine,
                    bass_nofuse=True,
                )
                nop.sync_info = bass_rust.SyncInfo(on_wait=[w], on_update=[])
                newlist.append(nop)
            si.on_wait = [keep]
            newlist.append(i)
        if len(newlist) != len(insts):
            insts[:] = newlist


def build_nc() -> bass.Bass:
    nc = bacc.Bacc()
    AL = mybir.AluOpType
    AF = mybir.ActivationFunctionType

    xTd = nc.declare_dram_parameter("xT", [128, NI, BC], F16, isOutput=False)
    WTd = nc.declare_dram_parameter("WT", [128, NE, NI, 128], F16, isOutput=False)
    W2d = nc.declare_dram_parameter("W2", [128, NE, IN], F16, isOutput=False)
    Cd = nc.declare_dram_parameter("consts", [128, NE, NCONST], F32, isOutput=False)
    Bd = nc.declare_dram_parameter("blast", [128, NI], F32, isOutput=False)
    Od = nc.declare_dram_parameter("outT", [128, NI, BC], F16, isOutput=True)

    with TileContext(nc) as tc:
        with (
            tc.tile_pool(name="persist", bufs=1) as persist,
            tc.tile_pool(name="wtp", bufs=4) as wtp,
            tc.tile_pool(name="w2p", bufs=3) as w2p,
            tc.tile_pool(name="tpool", bufs=4) as tpool,
            tc.tile_pool(name="upool", bufs=4) as upool,
            tc.tile_pool(name="xpool", bufs=4) as xpool,
            tc.tile_pool(name="hbpool", bufs=3) as hbpool,
            tc.tile_pool(name="ppool", bufs=3) as ppool,
            tc.tile_pool(name="outp", bufs=2) as outp,
            tc.tile_pool(name="psum_h", bufs=3, space="PSUM") as psum_h,
            tc.tile_pool(name="psum_o", bufs=2, space="PSUM") as psum_o,
        ):
            # PE p-state warmup: junk matmuls on a zeroed tile keep the PE
            # continuously busy through the initial DMA wait so real mm1
            # starts at full clock (ramp needs ~3us of back-to-back work).
            junk = persist.tile([128, 512], F16)
            nc.vector.memset(junk, 0.0)
            jp = psum_o.tile([128, 512], F32, tag="o", name="junk_ps")
            for _ in range(N_WARMUP_MM):
                nc.tensor.matmul(jp, junk[:, 0:128], junk, start=True, stop=True)
            consts_sb = persist.tile([128, NE, NCONST], F32)
            nc.sync.dma_start(out=consts_sb, in_=Cd[:, :, :])
            blast_sb = persist.tile([128, NI], F32)
            nc.sync.dma_start(out=blast_sb, in_=Bd[:, :])
            x_sb = persist.tile([128, NI, BC], F16)
            # x split across two DMA queues with the first weight tile
            # leading the scalar queue, so mm1 can start within ~4us
            pre_wt = {}
            pre_wt[0] = wtp.tile([128, NI, 128], F16, tag="wt", name="wt_pre0")
            nc.scalar.dma_start(out=pre_wt[0][:, 0:2, :], in_=WTd[:, 0, 0:2, :])
            nc.sync.dma_start(out=x_sb[:, 0, 0:512], in_=xTd[:, 0, 0:512])
            nc.scalar.dma_start(out=pre_wt[0][:, 2:, :], in_=WTd[:, 0, 2:, :])
            nc.sync.dma_start(out=x_sb[:, 0, 512:], in_=xTd[:, 0, 512:])
            for i in range(1, NI):
                q = nc.sync if i % 2 == 0 else nc.scalar
                q.dma_start(out=x_sb[:, i, :], in_=xTd[:, i, :])
            for ec in range(1, GROUP):
                wt = wtp.tile([128, NI, 128], F16, tag="wt")
                nc.sync.dma_start(out=wt, in_=WTd[:, ec, :, :])
                pre_wt[ec] = wt
            z_sb = persist.tile([128, NE, BC], F16)
            o_acc = persist.tile([128, NI, BC], F16)

            def c_ap(ec, col):
                return consts_sb[:, ec, col : col + 1]

            w2_tiles = {}

            def load_w2(ic, ec_lo, ec_hi):
                t = w2p.tile(
                    [128, 28, 128], F16, tag="w2", name=f"w2_{ic}_{ec_lo}"
                )
                nc.sync.dma_start(
                    out=t[:, : ec_hi - ec_lo, :],
                    in_=W2d[:, ec_lo:ec_hi, ic * 128 : (ic + 1) * 128],
                )
                w2_tiles[(ic, ec_lo)] = t

            def emit_mm2_accum(ops, ic, hf, ec_lo, ec_hi, w2_lo, start, stop):
                w2t = w2_tiles[(ic, w2_lo)]
                for ec in range(ec_lo, ec_hi):
                    nc.tensor.matmul(
                        ops,
                        w2t[:, ec - w2_lo, :],
                        z_sb[:, ec, hf * 512 : (hf + 1) * 512],
                        start=(start and ec == ec_lo),
                        stop=(stop and ec == ec_hi - 1),
                    )

            def emit_mm2_finish(ops, ic, hf, into_acc):
                bsl = hf * 512
                if into_acc:
                    # bias_last folded here; fp16 partial staging
                    nc.scalar.activation(
                        out=o_acc[:, ic, bsl : bsl + 512],
                        in_=ops,
                        func=AF.Identity,
                        bias=blast_sb[:, ic : ic + 1],
                        scale=1.0,
                    )
                else:
                    osb = outp.tile([128, 512], F16, tag="osb")
                    nc.vector.tensor_tensor(
                        out=osb, in0=o_acc[:, ic, bsl : bsl + 512], in1=ops,
                        op=AL.add,
                    )
                    nc.sync.dma_start(
                        out=Od[:, ic, bsl : bsl + 512], in_=osb
                    )

            def emit_mm2_group(ic, hf, ec_lo, ec_hi, into_acc):
                ops = psum_o.tile(
                    [128, 512], F32, tag="o", name=f"o_{ic}_{hf}_{ec_lo}"
                )
                emit_mm2_accum(ops, ic, hf, ec_lo, ec_hi, ec_lo, True, True)
                emit_mm2_finish(ops, ic, hf, into_acc)

            # window-1 whole-ic units spread over phase-A tail groups,
            # emitted AFTER each group's mm1 so they don't starve the cascade.
            w1_sched = {20: range(0, 3), 24: range(3, 5), 28: range(5, 8)}
            w1_depth = {}

            # ---------------- Phase A: mm1 + cascade ----------------
            for g0 in range(0, NE, GROUP):
                ecs = list(range(g0, g0 + GROUP))
                h_ps = {}
                for ec in ecs:
                    if ec in pre_wt:
                        wt = pre_wt[ec]
                    else:
                        wt = wtp.tile([128, NI, 128], F16, tag="wt")
                        nc.sync.dma_start(out=wt, in_=WTd[:, ec, :, :])
                    hp = psum_h.tile([128, BC], F32, tag="h")
                    for i in range(NI):
                        lhsT = wt[:, i, :]
                        for hf in range(2):
                            nc.tensor.matmul(
                                hp[:, hf * 512 : (hf + 1) * 512],
                                lhsT,
                                x_sb[:, i, hf * 512 : (hf + 1) * 512],
                                start=(i == 0),
                                stop=(i == NI - 1),
                            )
                    h_ps[ec] = hp

                for ic in w1_sched.get(g0, ()):
                    load_w2(ic, 0, g0)
                    w1_depth[ic] = g0
                    for hf in range(2):
                        emit_mm2_group(ic, hf, 0, g0, into_acc=True)

                pairs = [(ecs[0], ecs[1]), (ecs[2], ecs[3])]
                # ---- evict-affine: T = nu*h + mu on ACT ----
                T_t = {}
                for pi, (ea, eb) in enumerate(pairs):
                    t = tpool.tile([128, 2, BC], F16, tag="T", name=f"T_{ea}")
                    for j, ec in ((0, ea), (1, eb)):
                        nc.scalar.activation(
                            out=t[:, j, :],
                            in_=h_ps[ec],
                            func=AF.Identity,
                            bias=c_ap(ec, 1),
                            scale=c_ap(ec, 0),
                        )
                    T_t[pi] = t

                def emit_tt(k, pi, pg, in0, in1, out):
                    eng = nc.gpsimd if pg in TT_GP.get(k, ()) else nc.vector
                    eng.tensor_tensor(
                        out=out[:, :, :], in0=in0[:, :, :], in1=in1[:, :, :],
                        op=AL.add,
                    )

                # ---- cascade, layer-major across the 2 pairs ----
                x_cur = {}
                # L1 (always DVE): Ut_1 = ts(T, 0, t1; max, mult)
                for pi, (ea, eb) in enumerate(pairs):
                    pg = ea // 2
                    ut = upool.tile([128, 2, BC], F16, tag="u", name=f"u1_{ea}")
                    for j, ec in ((0, ea), (1, eb)):
                        nc.vector.tensor_scalar(
                            ut[:, j, :], T_t[pi][:, j, :],
                            0.0, c_ap(ec, 2), AL.max, AL.mult,
                        )
                    xt = xpool.tile([128, 2, BC], F16, tag="x", name=f"x2_{ea}")
                    emit_tt(1, pi, pg, T_t[pi], ut, xt)
                    x_cur[pi] = xt

                for k in range(2, L + 1):
                    base = 3 + 4 * (k - 2)
                    for pi, (ea, eb) in enumerate(pairs):
                        pg = ea // 2
                        X = x_cur[pi]
                        if pg in _act_pairs(k):
                            # ACT-impl: U = relu(a*X + b) on ACT
                            if k < L:
                                ut = upool.tile(
                                    [128, 2, BC], F16, tag="u", name=f"u{k}_{ea}"
                                )
                            for j, ec in ((0, ea), (1, eb)):
                                dst = z_sb[:, ec, :] if k == L else ut[:, j, :]
                                nc.scalar.activation(
                                    out=dst,
                                    in_=X[:, j, :],
                                    func=AF.Relu,
                                    bias=c_ap(ec, base + 1),
                                    scale=c_ap(ec, base + 0),
                                )
                            if k < L:
                                hbt = hbpool.tile(
                                    [128, 2, BC], F16, tag="hb", name=f"hb{k}_{ea}"
                                )
                                for j, ec in ((0, ea), (1, eb)):
                                    # 1-AP prescale; T's bias lands in phi
                                    nc.vector.tensor_scalar(
                                        hbt[:, j, :], T_t[pi][:, j, :],
                                        c_ap(ec, base + 2), None, AL.mult,
                                    )
                                xt = xpool.tile(
                                    [128, 2, BC], F16, tag="x", name=f"x{k+1}_{ea}"
                                )
                                emit_tt(k, pi, pg, hbt, ut, xt)
                                x_cur[pi] = xt
                        else:
                            # DVE-impl: P = ts(X, A, B); Ut = ts(P, 0, t)
                            pt = ppool.tile(
                                [128, 2, BC], F16, tag="p", name=f"p{k}_{ea}"
                            )
                            for j, ec in ((0, ea), (1, eb)):
                                nc.vector.tensor_scalar(
                                    pt[:, j, :], X[:, j, :],
                                    c_ap(ec, base + 0), c_ap(ec, base + 1),
                                    AL.mult, AL.add,
                                )
                            if k < L:
                                ut = upool.tile(
                                    [128, 2, BC], F16, tag="u", name=f"u{k}_{ea}"
                                )
                            for j, ec in ((0, ea), (1, eb)):
                                dst = z_sb[:, ec, :] if k == L else ut[:, j, :]
                                nc.vector.tensor_scalar(
                                    dst, pt[:, j, :],
                                    0.0, c_ap(ec, base + 2), AL.max, AL.mult,
                                )
                            if k < L:
                                xt = xpool.tile(
                                    [128, 2, BC], F16, tag="x", name=f"x{k+1}_{ea}"
                                )
                                emit_tt(k, pi, pg, T_t[pi], ut, xt)
                                x_cur[pi] = xt

            # ---------------- Phase B: mm2 window-2 + combine ----------------
            # Pre-wave: 4 shallow units start contracting [lo, 28) as soon as
            # z[:28] lands (groups 0-6 done), holding their PSUM tiles open;
            # the [28, 32) remainder + combine runs once the last group's z is
            # out. Remaining units run whole-range after, with psum_o bufs=4
            # keeping the tail matmul stream deep.
            units = sorted(
                [(ic, hf) for ic in range(NI) for hf in range(2)],
                key=lambda u: w1_depth[u[0]],
            )
            PRE = 28
            prewave = units[:2]
            pre_ops = {}
            for ic, hf in prewave:
                lo = w1_depth[ic]
                if (ic, lo) not in w2_tiles:
                    load_w2(ic, lo, NE)
                ops = psum_o.tile([128, 512], F32, tag="o", name=f"opre_{ic}_{hf}")
                emit_mm2_accum(ops, ic, hf, lo, PRE, lo, True, False)
                pre_ops[(ic, hf)] = ops
            for ic, hf in prewave:
                ops = pre_ops[(ic, hf)]
                emit_mm2_accum(ops, ic, hf, PRE, NE, w1_depth[ic], False, True)
                emit_mm2_finish(ops, ic, hf, into_acc=False)
            for ic, hf in units[2:]:
                lo = w1_depth[ic]
                if (ic, lo) not in w2_tiles:
                    load_w2(ic, lo, NE)
                emit_mm2_group(ic, hf, lo, NE, into_acc=False)

    nc.compile()
    return nc


def _prep_inputs(x, W, biases, bias_last, alpha, beta):
    """Host-side shard/relayout/constant precompute. Returns per-core in_maps."""
    x = np.asarray(x, np.float32)
    W = np.asarray(W, np.float32)
    biases = np.asarray(biases, np.float32)
    bias_last = np.asarray(bias_last, np.float32)
    alpha = np.asarray(alpha, np.float32)
    beta = np.asarray(beta, np.float32)

    sgn = lambda a: np.where(a >= 0, 1.0, -1.0).astype(np.float32)
    safe = lambda a: np.where(np.abs(a) < 1e-20, 1e-20, a)

    # true-recurrence consts: z_1 = ck0*h + ek0;
    # z_{k+1} = ck[k]*h + ek[k] + dk[k]*U_k (k=1..7); zf = alpha7*U_8
    ck = np.zeros((L, E), np.float32)
    ek = np.zeros((L, E), np.float32)
    dk = np.zeros((L, E), np.float32)
    ck[0] = beta[0]
    ek[0] = beta[0] * biases[0]
    for k in range(1, L):
        ck[k] = beta[k]
        ek[k] = beta[k] * biases[k]
        dk[k] = alpha[k - 1]

    # per-e impl masks from the pair tables
    eid = np.arange(E)
    pair_of_e = eid // 256
    act_e = {
        k: np.isin(pair_of_e, list(_act_pairs(k))) for k in range(2, L + 1)
    }

    # rho1 normalization: bound |nu/ck[k]| over DVE-impl TT layers
    worst = np.ones(E, np.float32)
    for k in range(1, L):
        dve = np.ones(E, bool) if k == 1 else ~act_e[k]
        w = np.where(dve, np.abs(ck[0] / safe(ck[k])), 1.0)
        worst = np.maximum(worst, w.astype(np.float32))
    rho1 = (1.0 / np.maximum(1.0, worst / 64.0)).astype(np.float32)

    nu = rho1 * ck[0]
    mu = rho1 * ek[0]

    consts = np.zeros((E, NCONST), np.float32)
    consts[:, 0] = nu
    consts[:, 1] = mu

    # L1 (always DVE): t1 = (nu/ck1)*dk1/rho1
    rho_nD = nu / safe(ck[1])
    consts[:, 2] = rho_nD * dk[1] / rho1
    rho = rho_nD.copy()
    phi = mu - rho * ek[1]

    for k in range(2, L + 1):
        base = 3 + 4 * (k - 2)
        am = act_e[k]
        rho_hat = np.abs(rho)
        s_rho = sgn(rho)
        if k == L:
            lam8 = np.abs(alpha[L - 1])
            # ACT: a, b
            aA = lam8 / rho
            consts[:, base + 0] = np.where(am, aA, s_rho)
            consts[:, base + 1] = np.where(am, -aA * phi, -s_rho * phi)
            consts[:, base + 2] = np.where(am, 0.0, lam8 / rho_hat)
            break
        lam = np.abs(dk[k])
        s = sgn(dk[k])
        # ACT path consts (hb is a 1-slot mult; its missing bias term
        # A_hb*mu - s*ek lands in phi and is corrected downstream)
        aA = lam / rho
        bA = -aA * phi
        A_hb = s * ck[k] / safe(nu)
        # DVE path consts
        A_P = s_rho
        B_P = -A_P * phi
        rho_nD = nu / safe(ck[k])
        t_k = rho_nD * dk[k] / rho_hat
        consts[:, base + 0] = np.where(am, aA, A_P)
        consts[:, base + 1] = np.where(am, bA, B_P)
        consts[:, base + 2] = np.where(am, A_hb, t_k)
        rho = np.where(am, s, rho_nD).astype(np.float32)
        phi = np.where(
            am, A_hb * mu - s * ek[k], mu - rho_nD * ek[k]
        ).astype(np.float32)

    consts_t = np.ascontiguousarray(
        consts.reshape(NE, 128, NCONST).transpose(1, 0, 2)
    )

    WT_t = np.ascontiguousarray(
        W.T.reshape(NI, 128, NE, 128).transpose(1, 2, 0, 3).astype(np.float16)
    )
    W2 = W * sgn(alpha[L - 1])[:, None]
    W2_t = np.ascontiguousarray(
        W2.reshape(NE, 128, IN).transpose(1, 0, 2).astype(np.float16)
    )
    blast_t = np.ascontiguousarray(bias_last.reshape(NI, 128).T)

    in_maps = []
    for c in range(N_CORES):
        xc = x[c * BC : (c + 1) * BC]           # (BC, IN)
        xT = np.ascontiguousarray(
            xc.T.reshape(NI, 128, BC).transpose(1, 0, 2).astype(np.float16)
        )
        in_maps.append(
            {
                "xT": xT,
                "WT": WT_t,
                "W2": W2_t,
                "consts": consts_t,
                "blast": blast_t,
            }
        )
    return in_maps


_NC_CACHE = None


def _install_ntff_hook():
    """The agent image's antenv lacks axon_hooks; rebuild it from the boot
    helper so run_bass_kernel_spmd(trace=True) can capture NTFF profiles."""
    import sys
    import types

    if "antenv.axon_hooks" in sys.modules:
        return
    try:
        from trn_agent_boot.trn_boot import _ntff_profile_via_ctypes

        hook = _ntff_profile_via_ctypes("/opt/axon/libaxon_pjrt.so")
    except Exception:
        hook = None
    m = types.ModuleType("antenv.axon_hooks")
    m.get_axon_ntff_profile_hook = lambda: hook
    m.set_axon_ntff_profile_hook = lambda h: None
    sys.modules["antenv.axon_hooks"] = m


def run(inputs: dict, trace: bool = False):
    """Returns (out, BassKernelResults)."""
    global _NC_CACHE
    from concourse.bass_utils import run_bass_kernel_spmd

    if trace:
        _install_ntff_hook()

    if _NC_CACHE is None:
        _NC_CACHE = build_nc()
    nc = _NC_CACHE
    in_maps = _prep_inputs(**inputs)
    res = run_bass_kernel_spmd(nc, in_maps, list(range(N_CORES)), trace=trace)
    out = np.empty((B, IN), np.float32)
    for c in range(N_CORES):
        oc = np.asarray(res.results[c]["outT"]).astype(np.float32)
        out_core = oc.transpose(1, 0, 2).reshape(IN, BC) # (IN, BC) = outT
        out[c * BC : (c + 1) * BC] = out_core.T
    return out, res


def kernel(x, W, biases, bias_last, alpha, beta) -> np.ndarray:
    out, _ = run(
        dict(x=x, W=W, biases=biases, bias_last=bias_last, alpha=alpha, beta=beta)
    )
    return out
